# revision 8
# baseline (speedup 1.0000x reference)
"""Trainium2 Bass kernel for AvgReadout-style segment mean + L2 normalize.

reference:
    vsum[i] = sum over edges e with src[e]==i of emb[dst[e]]
    deg[i]  = count of such edges (clamped to >=1)
    out     = l2_normalize(vsum / deg, eps=1e-12)

Key identity: l2_normalize(vsum/deg) == l2_normalize(vsum) whenever deg >= 1
(positive per-row scalar doesn't change direction), and for deg == 0 both are
exactly 0.  So the kernel only needs vsum, never deg.

Distribution: edges are sorted by src on host and sharded by src-range across
8 cores (12500 segments each).  Each core's output slice is disjoint, so no
collectives are needed.

Per core the 12500 segments form 98 blocks of 128, processed in superblocks
of SB=4 blocks (4 concurrent PSUM tiles).  Edge rows are fetched with
dma_gather (int16 indices force 4 quarter tables of 25000 emb rows).  Edges
are bucketed into cells (superblock, quarter) and padded to whole subtiles of
128 edges; cell capacities are maxed across cores so one compiled program
serves all 8 cores (measured padding ~6.5%; per-block cells would cost 25%).
The gather is HBM-random-access-bound (~2.4ns/row on HW), so total gathered
slot count is the dominant cost; pad indices are spread across the quarter
because repeated fetches of one row serialize on an HBM bank (measured 2.7x
slowdown when all indices equal).

Within a cell, edges are sorted by segment, so a subtile usually holds edges
of one block and spans two at block transitions.  Each slot carries a bitmask
of rel-blocks present (union across cores).  Per (gather call, rel-block) one
batched one-hot build on DVE compares srcloc (s mod 512) against an iota
slice offset by 128*rb; the layout is (seg-major, subtile-minor) so every
DVE operand has a packed innermost dim, enabling the DVE 2x 16-bit mode.
Edges of other blocks mismatch and contribute zero columns.  The PE matmul
for block sb0+rb accumulates lhsT = one-hot columns (stride CALL_CAP) x
rhs = gathered rows into the block's PSUM tile [128 segs, 128 feat].  Pad
edges carry an out-of-range srcloc sentinel (all-zero one-hot column); their
fetched rows are real emb rows, so everything stays finite.

Epilogue per block: sum-of-squares (ACT Square+accum), sqrt, clamp 1e-12,
reciprocal, scale-copy, DMA out.

build_program(repeats=N) unrolls the body N times in one NEFF; hw_loop=U
instead wraps N/U iterations of U unrolled bodies in a tc.For_i hardware
loop (constant compile time; the per-iteration all-engine barrier is
amortized over U bodies).  test.py uses that to amortize the ~60ms axon
dispatch floor out of the per-iteration timing.
"""

import numpy as np
from contextlib import ExitStack

N_SPOT = 100000
D = 128
P = 128
NCORES = 8
SEG_PER_CORE = 12500
NBLK = (SEG_PER_CORE + P - 1) // P  # 98
NQ = 4                 # emb quarter tables (int16 index range)
QROWS = N_SPOT // NQ   # 25000
SB = 4                 # blocks per superblock == blocks per cell
NSB = (NBLK + SB - 1) // SB  # 25
CALL_CAP = 8           # subtiles per dma_gather call
NQUEUES = 4            # SWDGE queues to round-robin
RING = 16384           # dynamic DMA scratch bytes (1024 descriptor ring)
PAD_SENTINEL = 3000.0  # outside [0, SB*P): pad edges match no one-hot column


def preprocess(emb, mask, call_cap=CALL_CAP):
    """Sort/shard/pad edges. Returns (in_maps, capsub, layout)."""
    qrows = QROWS
    emb = np.ascontiguousarray(np.asarray(emb, dtype=np.float32))
    emb16 = emb.astype(np.float16)
    mask = np.asarray(mask)
    src = mask[0].astype(np.int64, copy=False)
    dst = mask[1].astype(np.int64, copy=False)

    order = np.argsort(src, kind="stable")
    src_s = src[order].astype(np.int32)
    dst_s = dst[order].astype(np.int32)

    core_bounds = np.searchsorted(
        src_s, (SEG_PER_CORE * np.arange(NCORES + 1)).astype(np.int32)
    )

    ncell = NSB * NQ
    percore = []
    cnts = np.zeros((NCORES, ncell), np.int64)
    for k in range(NCORES):
        lo, hi = int(core_bounds[k]), int(core_bounds[k + 1])
        s = src_s[lo:hi] - SEG_PER_CORE * k
        d = dst_s[lo:hi]
        cell = (s >> 9) * NQ + d // qrows
        # sort by (cell, rel-block, d): rel-block grouping keeps subtile
        # spanning minimal; d-order within a block keeps the HBM gather
        # access pattern local (the gather is HBM-random-bound)
        o = np.lexsort((d, s >> 7, cell))
        s, d, cell = s[o], d[o], cell[o]
        cnts[k] = np.bincount(cell, minlength=ncell)
        percore.append((s, d, cell))

    valid_f = cnts.max(axis=0)  # [ncell] cross-core max edge count
    capsub = (-(-valid_f // P)).astype(np.int64)  # [ncell] subtiles
    nslots = int(capsub.sum())

    cell_slot0 = np.zeros(ncell, np.int64)
    cell_slot0[1:] = np.cumsum(capsub)[:-1]
    cell_base = cell_slot0 * P

    slot_mask = np.zeros(nslots, np.int64)  # rel-block bitmask, cross-core union

    # iota[p, (c*call_cap) + t] = c for c in [0, SB*P)
    iota512 = np.broadcast_to(
        np.repeat(np.arange(SB * P), call_cap).astype(np.float16)[None, :],
        (P, SB * P * call_cap),
    ).copy()

    # spread pad gather indices across the quarter: repeated fetches of a
    # single row serialize on an HBM bank (measured 2.7x slowdown)
    pad_spread = ((np.arange(nslots * P, dtype=np.int64) * 97) % qrows).astype(
        np.int16
    )

    in_maps = []
    for k in range(NCORES):
        s, d, cell = percore[k]
        cum = np.zeros(ncell, np.int64)
        cc = cnts[k]
        cum[1:] = np.cumsum(cc)[:-1]
        rank = np.arange(len(s), dtype=np.int64) - cum[cell]
        pos = cell_base[cell] + rank

        slot_g = cell_slot0[cell] + (rank >> 7)
        rb = (s >> 7) & (SB - 1)
        np.bitwise_or.at(slot_mask, slot_g, 1 << rb)

        srcloc = np.full(nslots * P, PAD_SENTINEL, np.float16)
        srcloc[pos] = (s & (SB * P - 1)).astype(np.float16)
        dloc = pad_spread.copy()
        dloc[pos] = (d % qrows).astype(np.int16)

        # -1 tail beyond the cell's valid count: the Q7 ucode skips those
        # descriptors entirely (no HBM fetch).  [cnt_k, valid) keeps spread
        # dummy indices so every core's non-negative count == the static
        # num_idxs_reg.  Skipped partitions keep stale SBUF, kept finite by
        # the gather-buffer memset prologue in build_program.
        tail = (capsub * P - valid_f).astype(np.int64)
        if tail.sum():
            cells_t = np.repeat(np.arange(ncell), tail)
            offs_t = np.arange(len(cells_t)) - np.repeat(
                np.cumsum(tail) - tail, tail
            )
            tpos = cell_base[cells_t] + valid_f[cells_t] + offs_t
            dloc[tpos] = -1

        srcloc_t = np.ascontiguousarray(srcloc.reshape(nslots, P).T)
        # idx16 [j%16, slot*8 + j//16] = dloc of edge (slot, j), replicated
        # across the 8 partition groups for the Q7 ucode.
        idx_blk = np.ascontiguousarray(dloc.reshape(nslots * 8, 16).T)
        idx16 = np.tile(idx_blk, (8, 1))
        in_maps.append(
            {"emb": emb16, "srcloc": srcloc_t, "dstidx": idx16, "iota": iota512}
        )

    # layout: gather calls + per-call rb runs + per-block matmul lists
    calls = []           # (q, slot0, nsub, reg)
    call_rb_runs = []    # per call: [(rb, t_lo, t_len)]
    blk_matmuls = [[] for _ in range(NBLK)]  # (call_idx, t, rb)
    sb_list = []         # (blocks, (call_lo, call_hi))
    for isb in range(NSB):
        blocks = list(range(isb * SB, min((isb + 1) * SB, NBLK)))
        call_lo = len(calls)
        for q in range(NQ):
            c = isb * NQ + q
            s0c = int(cell_slot0[c])
            cap = int(capsub[c])
            for i in range(0, cap, call_cap):
                nsub = min(call_cap, cap - i)
                s0 = s0c + i
                ci = len(calls)
                reg = max(1, min(int(valid_f[c]) - i * P, nsub * P))
                calls.append((q, s0, nsub, reg))
                runs = []
                for rb in range(SB):
                    ts = [t for t in range(nsub)
                          if slot_mask[s0 + t] & (1 << rb)]
                    if not ts:
                        continue
                    # split into contiguous stretches (cross-core union of
                    # per-core contiguous ranges can, in principle, have gaps)
                    t_lo = ts[0]
                    prev = ts[0]
                    for t in ts[1:] + [None]:
                        if t is not None and t == prev + 1:
                            prev = t
                            continue
                        runs.append((rb, t_lo, prev - t_lo + 1))
                        if t is not None:
                            t_lo = prev = t
                    b = isb * SB + rb
                    for t in ts:
                        blk_matmuls[b].append((ci, t, rb))
                call_rb_runs.append(runs)
        sb_list.append((blocks, (call_lo, len(calls))))

    layout = {
        "nslots": nslots,
        "calls": calls,
        "call_rb_runs": call_rb_runs,
        "blk_matmuls": blk_matmuls,
        "sb_list": sb_list,
    }
    return in_maps, capsub, layout


def build_program(capsub, layout, repeats=1, call_cap=CALL_CAP, ring=RING,
                  gbufs=28, obufs=36, hw_loop=False):
    import concourse.bass as bass
    import concourse.tile as tile
    from concourse import bacc, mybir

    qrows = QROWS
    nslots = layout["nslots"]
    calls = layout["calls"]
    call_rb_runs = layout["call_rb_runs"]
    blk_matmuls = layout["blk_matmuls"]
    sb_list = layout["sb_list"]
    d = D

    nc = bacc.Bacc(
        "TRN2", target_bir_lowering=False, debug=False,
        num_swdge_queues=NQUEUES, dynamic_dma_scratch_size=ring,
    )
    emb_t = nc.dram_tensor("emb", [N_SPOT, d], mybir.dt.float16, kind="ExternalInput")
    srcloc_t = nc.dram_tensor(
        "srcloc", [P, nslots], mybir.dt.float16, kind="ExternalInput"
    )
    dstidx_t = nc.dram_tensor(
        "dstidx", [P, nslots * 8], mybir.dt.int16, kind="ExternalInput"
    )
    iota_t = nc.dram_tensor("iota", [P, SB * P * call_cap], mybir.dt.float16,
                            kind="ExternalInput")
    out_t = nc.dram_tensor(
        "out", [NBLK * P, d], mybir.dt.float32, kind="ExternalOutput"
    )

    with tile.TileContext(nc) as tc, ExitStack() as ctx:
        consts = ctx.enter_context(tc.tile_pool(name="consts", bufs=1))
        gpool = ctx.enter_context(tc.tile_pool(name="gather", bufs=gbufs))
        ohpool = ctx.enter_context(tc.tile_pool(name="onehot", bufs=obufs))
        spool = ctx.enter_context(tc.tile_pool(name="scratch", bufs=4))
        opool = ctx.enter_context(tc.tile_pool(name="outs", bufs=4))
        ppool = ctx.enter_context(tc.tile_pool(name="psum", bufs=8, space="PSUM"))

        srcloc_sb = consts.tile([P, nslots], mybir.dt.float16)
        nc.sync.dma_start(srcloc_sb[:], srcloc_t.ap())
        dstidx_sb = consts.tile([P, nslots * 8], mybir.dt.int16)
        nc.sync.dma_start(dstidx_sb[:], dstidx_t.ap())
        iota_sb = consts.tile([P, SB * P * call_cap], mybir.dt.float16)
        nc.sync.dma_start(iota_sb[:], iota_t.ap())

        out_ap = out_t.ap()
        emb_ap = emb_t.ap()

        # zero every gather buffer once: calls with a -1 index tail skip
        # those descriptors, leaving stale SBUF in the tail partitions;
        # it must be finite (NaN * 0 one-hot = NaN would poison the psum)
        for _ in range(gbufs):
            warm = gpool.tile([P, call_cap * d], mybir.dt.float16, tag="gt")
            nc.vector.memset(warm[:], 0.0)

        def emit_body():
            callno = 0
            for blocks, (clo, chi) in sb_list:
                gtiles = {}
                ohtiles = {}
                for ci in range(clo, chi):
                    q, s0, nsub, reg = calls[ci]
                    gt = gpool.tile([P, call_cap * d], mybir.dt.float16, tag="gt")
                    nc.gpsimd.dma_gather(
                        out_ap=gt[:, : nsub * d].rearrange(
                            "p (c e) -> p c e", e=d
                        ),
                        in_ap=emb_ap[q * qrows : (q + 1) * qrows, :],
                        idxs_ap=dstidx_sb[:, s0 * 8 : (s0 + nsub) * 8],
                        num_idxs=nsub * P,
                        num_idxs_reg=reg,
                        elem_size=d,
                        single_packet=False,
                        queue_num=callno % NQUEUES,
                    )
                    gtiles[ci] = gt
                    callno += 1
                    iota_full = iota_sb[:, :]
                    srl0 = srcloc_sb[:, s0 : s0 + nsub]
                    for rb, t_lo, t_len in call_rb_runs[ci]:
                        oh = ohpool.tile([P, call_cap * P], mybir.dt.float16,
                                         tag="oh")
                        full = oh[:, :]
                        oh3 = bass.AP(
                            full.tensor, full.offset + t_lo,
                            [full.ap[0], [call_cap, P], [1, t_len]],
                        )
                        iota_b = bass.AP(
                            iota_full.tensor,
                            iota_full.offset + rb * P * call_cap + t_lo,
                            [iota_full.ap[0], [call_cap, P], [1, t_len]],
                        )
                        srl_b = bass.AP(
                            srl0.tensor, srl0.offset + t_lo,
                            [srl0.ap[0], [0, P], [1, t_len]],
                        )
                        nc.vector.tensor_tensor(
                            out=oh3, in0=iota_b, in1=srl_b,
                            op=mybir.AluOpType.is_equal,
                        )
                        ohtiles[(ci, rb)] = oh
                for b in blocks:
                    mms = blk_matmuls[b]
                    if not mms:
                        ot = opool.tile([P, d], mybir.dt.float32)
                        nc.vector.memset(ot[:], 0.0)
                        nc.sync.dma_start(out_ap[b * P : (b + 1) * P, :], ot[:])
                        continue
                    ps = ppool.tile([P, d], mybir.dt.float32, space="PSUM")
                    for i, (ci, t, rb) in enumerate(mms):
                        ohfull = ohtiles[(ci, rb)][:, :]
                        lhsT = bass.AP(
                            ohfull.tensor, ohfull.offset + t,
                            [ohfull.ap[0], [call_cap, P]],
                        )
                        nc.tensor.matmul(
                            ps[:],
                            lhsT=lhsT,
                            rhs=gtiles[ci][:, t * d : (t + 1) * d],
                            start=(i == 0),
                            stop=(i == len(mms) - 1),
                        )
                    sq = spool.tile([P, d], mybir.dt.float32)
                    ss = spool.tile([P, 1], mybir.dt.float32)
                    nc.scalar.activation(
                        sq[:], ps[:], mybir.ActivationFunctionType.Square,
                        accum_out=ss[:],
                    )
                    nrm = spool.tile([P, 1], mybir.dt.float32)
                    nc.scalar.activation(
                        nrm[:], ss[:], mybir.ActivationFunctionType.Sqrt
                    )
                    nc.vector.tensor_scalar(
                        out=nrm[:], in0=nrm[:], scalar1=1e-12, scalar2=None,
                        op0=mybir.AluOpType.max,
                    )
                    nc.vector.reciprocal(nrm[:], nrm[:])
                    ot = opool.tile([P, d], mybir.dt.float32)
                    nc.scalar.activation(
                        ot[:], ps[:], mybir.ActivationFunctionType.Copy,
                        scale=nrm[:],
                    )
                    nc.sync.dma_start(out_ap[b * P : (b + 1) * P, :], ot[:])

        if hw_loop and repeats > 1:
            assert repeats % hw_loop == 0
            with tc.For_i(0, repeats // hw_loop) as _i:
                for _u in range(hw_loop):
                    emit_body()
        else:
            for _rep in range(repeats):
                emit_body()

    nc.compile()
    return nc


_PROGRAM_CACHE = {}


def _get_program(capsub, layout, **kw):
    key = (capsub.tobytes(), tuple(sorted(kw.items())))
    if key not in _PROGRAM_CACHE:
        _PROGRAM_CACHE[key] = build_program(capsub, layout, **kw)
    return _PROGRAM_CACHE[key]


def kernel(**inputs):
    emb = inputs["emb"]
    mask = inputs["mask"]
    in_maps, capsub, layout = preprocess(emb, mask)
    nc = _get_program(capsub, layout)

    from concourse.bass_utils import run_bass_kernel_spmd

    res = run_bass_kernel_spmd(nc, in_maps, core_ids=list(range(NCORES)))
    out = np.empty((N_SPOT, D), np.float32)
    for k in range(NCORES):
        out[k * SEG_PER_CORE : (k + 1) * SEG_PER_CORE] = res.results[k]["out"][
            :SEG_PER_CORE
        ]
    return out


# revision 9
# speedup vs baseline: 1.0066x; 1.0066x over previous
"""Trainium2 Bass kernel for AvgReadout-style segment mean + L2 normalize.

reference:
    vsum[i] = sum over edges e with src[e]==i of emb[dst[e]]
    deg[i]  = count of such edges (clamped to >=1)
    out     = l2_normalize(vsum / deg, eps=1e-12)

Key identity: l2_normalize(vsum/deg) == l2_normalize(vsum) whenever deg >= 1
(positive per-row scalar doesn't change direction), and for deg == 0 both are
exactly 0.  So the kernel only needs vsum, never deg.

Distribution: edges are sorted by src on host and sharded by src-range across
8 cores (12500 segments each).  Each core's output slice is disjoint, so no
collectives are needed.

Per core the 12500 segments form 98 blocks of 128, processed in superblocks
of SB=4 blocks (4 concurrent PSUM tiles).  Edge rows are fetched with
dma_gather (int16 indices force 4 quarter tables of 25000 emb rows).  Edges
are bucketed into cells (superblock, quarter) and padded to whole subtiles of
128 edges; cell capacities are maxed across cores so one compiled program
serves all 8 cores (measured padding ~6.5%; per-block cells would cost 25%).
The gather is HBM-random-access-bound (~2.4ns/row on HW), so total gathered
slot count is the dominant cost; pad indices are spread across the quarter
because repeated fetches of one row serialize on an HBM bank (measured 2.7x
slowdown when all indices equal).

Within a cell, edges are sorted by segment, so a subtile usually holds edges
of one block and spans two at block transitions.  Each slot carries a bitmask
of rel-blocks present (union across cores).  Per (gather call, rel-block) one
batched one-hot build on DVE compares srcloc (s mod 512) against an iota
slice offset by 128*rb; the layout is (seg-major, subtile-minor) so every
DVE operand has a packed innermost dim, enabling the DVE 2x 16-bit mode.
Edges of other blocks mismatch and contribute zero columns.  The PE matmul
for block sb0+rb accumulates lhsT = one-hot columns (stride CALL_CAP) x
rhs = gathered rows into the block's PSUM tile [128 segs, 128 feat].  Pad
edges carry an out-of-range srcloc sentinel (all-zero one-hot column); their
fetched rows are real emb rows, so everything stays finite.

Epilogue per block: sum-of-squares (ACT Square+accum), sqrt, clamp 1e-12,
reciprocal, scale-copy, DMA out.

build_program(repeats=N) unrolls the body N times in one NEFF; hw_loop=U
instead wraps N/U iterations of U unrolled bodies in a tc.For_i hardware
loop (constant compile time; the per-iteration all-engine barrier is
amortized over U bodies).  test.py uses that to amortize the ~60ms axon
dispatch floor out of the per-iteration timing.
"""

import numpy as np
from contextlib import ExitStack

N_SPOT = 100000
D = 128
P = 128
NCORES = 8
SEG_PER_CORE = 12500
NBLK = (SEG_PER_CORE + P - 1) // P  # 98
NQ = 4                 # emb quarter tables (int16 index range)
QROWS = N_SPOT // NQ   # 25000
SB = 4                 # blocks per superblock == blocks per cell
NSB = (NBLK + SB - 1) // SB  # 25
CALL_CAP = 8           # subtiles per dma_gather call
NQUEUES = 4            # SWDGE queues to round-robin
RING = 16384           # dynamic DMA scratch bytes (1024 descriptor ring)
PAD_SENTINEL = 3000.0  # outside [0, SB*P): pad edges match no one-hot column


def preprocess(emb, mask, call_cap=CALL_CAP):
    """Sort/shard/pad edges. Returns (in_maps, capsub, layout)."""
    qrows = QROWS
    emb = np.ascontiguousarray(np.asarray(emb, dtype=np.float32))
    emb16 = emb.astype(np.float16)
    mask = np.asarray(mask)
    src = mask[0].astype(np.int64, copy=False)
    dst = mask[1].astype(np.int64, copy=False)

    order = np.argsort(src, kind="stable")
    src_s = src[order].astype(np.int32)
    dst_s = dst[order].astype(np.int32)

    core_bounds = np.searchsorted(
        src_s, (SEG_PER_CORE * np.arange(NCORES + 1)).astype(np.int32)
    )

    ncell = NSB * NQ
    percore = []
    cnts = np.zeros((NCORES, ncell), np.int64)
    for k in range(NCORES):
        lo, hi = int(core_bounds[k]), int(core_bounds[k + 1])
        s = src_s[lo:hi] - SEG_PER_CORE * k
        d = dst_s[lo:hi]
        cell = (s >> 9) * NQ + d // qrows
        # sort by (cell, rel-block, d): rel-block grouping keeps subtile
        # spanning minimal; d-order within a block keeps the HBM gather
        # access pattern local (the gather is HBM-random-bound)
        o = np.lexsort((d, s >> 7, cell))
        s, d, cell = s[o], d[o], cell[o]
        cnts[k] = np.bincount(cell, minlength=ncell)
        percore.append((s, d, cell))

    capsub = (-(-cnts.max(axis=0) // P)).astype(np.int64)  # [ncell] subtiles
    nslots = int(capsub.sum())

    cell_slot0 = np.zeros(ncell, np.int64)
    cell_slot0[1:] = np.cumsum(capsub)[:-1]
    cell_base = cell_slot0 * P

    slot_mask = np.zeros(nslots, np.int64)  # rel-block bitmask, cross-core union

    # iota[p, (c*call_cap) + t] = c for c in [0, SB*P)
    iota512 = np.broadcast_to(
        np.repeat(np.arange(SB * P), call_cap).astype(np.float16)[None, :],
        (P, SB * P * call_cap),
    ).copy()

    # spread pad gather indices across the quarter: repeated fetches of a
    # single row serialize on an HBM bank (measured 2.7x slowdown)
    pad_spread = ((np.arange(nslots * P, dtype=np.int64) * 97) % qrows).astype(
        np.int16
    )

    in_maps = []
    for k in range(NCORES):
        s, d, cell = percore[k]
        cum = np.zeros(ncell, np.int64)
        cc = cnts[k]
        cum[1:] = np.cumsum(cc)[:-1]
        rank = np.arange(len(s), dtype=np.int64) - cum[cell]
        pos = cell_base[cell] + rank

        slot_g = cell_slot0[cell] + (rank >> 7)
        rb = (s >> 7) & (SB - 1)
        np.bitwise_or.at(slot_mask, slot_g, 1 << rb)

        srcloc = np.full(nslots * P, PAD_SENTINEL, np.float16)
        srcloc[pos] = (s & (SB * P - 1)).astype(np.float16)
        dloc = pad_spread.copy()
        dloc[pos] = (d % qrows).astype(np.int16)

        srcloc_t = np.ascontiguousarray(srcloc.reshape(nslots, P).T)
        # idx16 [j%16, slot*8 + j//16] = dloc of edge (slot, j), replicated
        # across the 8 partition groups for the Q7 ucode.
        idx_blk = np.ascontiguousarray(dloc.reshape(nslots * 8, 16).T)
        idx16 = np.tile(idx_blk, (8, 1))
        in_maps.append(
            {"emb": emb16, "srcloc": srcloc_t, "dstidx": idx16, "iota": iota512}
        )

    # layout: gather calls + per-call rb runs + per-block matmul lists
    calls = []           # (q, slot0, nsub)
    call_rb_runs = []    # per call: [(rb, t_lo, t_len)]
    blk_matmuls = [[] for _ in range(NBLK)]  # (call_idx, t, rb)
    sb_list = []         # (blocks, (call_lo, call_hi))
    for isb in range(NSB):
        blocks = list(range(isb * SB, min((isb + 1) * SB, NBLK)))
        call_lo = len(calls)
        for q in range(NQ):
            c = isb * NQ + q
            s0c = int(cell_slot0[c])
            cap = int(capsub[c])
            for i in range(0, cap, call_cap):
                nsub = min(call_cap, cap - i)
                s0 = s0c + i
                ci = len(calls)
                calls.append((q, s0, nsub))
                runs = []
                for rb in range(SB):
                    ts = [t for t in range(nsub)
                          if slot_mask[s0 + t] & (1 << rb)]
                    if not ts:
                        continue
                    # split into contiguous stretches (cross-core union of
                    # per-core contiguous ranges can, in principle, have gaps)
                    t_lo = ts[0]
                    prev = ts[0]
                    for t in ts[1:] + [None]:
                        if t is not None and t == prev + 1:
                            prev = t
                            continue
                        runs.append((rb, t_lo, prev - t_lo + 1))
                        if t is not None:
                            t_lo = prev = t
                    b = isb * SB + rb
                    for t in ts:
                        blk_matmuls[b].append((ci, t, rb))
                call_rb_runs.append(runs)
        sb_list.append((blocks, (call_lo, len(calls))))

    layout = {
        "nslots": nslots,
        "calls": calls,
        "call_rb_runs": call_rb_runs,
        "blk_matmuls": blk_matmuls,
        "sb_list": sb_list,
    }
    return in_maps, capsub, layout


def build_program(capsub, layout, repeats=1, call_cap=CALL_CAP, ring=RING,
                  gbufs=28, obufs=36, hw_loop=False):
    import concourse.bass as bass
    import concourse.tile as tile
    from concourse import bacc, mybir

    qrows = QROWS
    nslots = layout["nslots"]
    calls = layout["calls"]
    call_rb_runs = layout["call_rb_runs"]
    blk_matmuls = layout["blk_matmuls"]
    sb_list = layout["sb_list"]
    d = D

    nc = bacc.Bacc(
        "TRN2", target_bir_lowering=False, debug=False,
        num_swdge_queues=NQUEUES, dynamic_dma_scratch_size=ring,
    )
    emb_t = nc.dram_tensor("emb", [N_SPOT, d], mybir.dt.float16, kind="ExternalInput")
    srcloc_t = nc.dram_tensor(
        "srcloc", [P, nslots], mybir.dt.float16, kind="ExternalInput"
    )
    dstidx_t = nc.dram_tensor(
        "dstidx", [P, nslots * 8], mybir.dt.int16, kind="ExternalInput"
    )
    iota_t = nc.dram_tensor("iota", [P, SB * P * call_cap], mybir.dt.float16,
                            kind="ExternalInput")
    out_t = nc.dram_tensor(
        "out", [NBLK * P, d], mybir.dt.float32, kind="ExternalOutput"
    )

    with tile.TileContext(nc) as tc, ExitStack() as ctx:
        consts = ctx.enter_context(tc.tile_pool(name="consts", bufs=1))
        gpool = ctx.enter_context(tc.tile_pool(name="gather", bufs=gbufs))
        ohpool = ctx.enter_context(tc.tile_pool(name="onehot", bufs=obufs))
        spool = ctx.enter_context(tc.tile_pool(name="scratch", bufs=4))
        opool = ctx.enter_context(tc.tile_pool(name="outs", bufs=4))
        ppool = ctx.enter_context(tc.tile_pool(name="psum", bufs=8, space="PSUM"))

        srcloc_sb = consts.tile([P, nslots], mybir.dt.float16)
        nc.sync.dma_start(srcloc_sb[:], srcloc_t.ap())
        dstidx_sb = consts.tile([P, nslots * 8], mybir.dt.int16)
        nc.sync.dma_start(dstidx_sb[:], dstidx_t.ap())
        iota_sb = consts.tile([P, SB * P * call_cap], mybir.dt.float16)
        nc.sync.dma_start(iota_sb[:], iota_t.ap())

        out_ap = out_t.ap()
        emb_ap = emb_t.ap()

        def emit_body():
            callno = 0
            for blocks, (clo, chi) in sb_list:
                gtiles = {}
                ohtiles = {}
                for ci in range(clo, chi):
                    q, s0, nsub = calls[ci]
                    gt = gpool.tile([P, call_cap * d], mybir.dt.float16, tag="gt")
                    nc.gpsimd.dma_gather(
                        out_ap=gt[:, : nsub * d].rearrange(
                            "p (c e) -> p c e", e=d
                        ),
                        in_ap=emb_ap[q * qrows : (q + 1) * qrows, :],
                        idxs_ap=dstidx_sb[:, s0 * 8 : (s0 + nsub) * 8],
                        num_idxs=nsub * P,
                        num_idxs_reg=nsub * P,
                        elem_size=d,
                        single_packet=False,
                        queue_num=callno % NQUEUES,
                    )
                    gtiles[ci] = gt
                    callno += 1
                    iota_full = iota_sb[:, :]
                    srl0 = srcloc_sb[:, s0 : s0 + nsub]
                    for rb, t_lo, t_len in call_rb_runs[ci]:
                        oh = ohpool.tile([P, call_cap * P], mybir.dt.float16,
                                         tag="oh")
                        full = oh[:, :]
                        oh3 = bass.AP(
                            full.tensor, full.offset + t_lo,
                            [full.ap[0], [call_cap, P], [1, t_len]],
                        )
                        iota_b = bass.AP(
                            iota_full.tensor,
                            iota_full.offset + rb * P * call_cap + t_lo,
                            [iota_full.ap[0], [call_cap, P], [1, t_len]],
                        )
                        srl_b = bass.AP(
                            srl0.tensor, srl0.offset + t_lo,
                            [srl0.ap[0], [0, P], [1, t_len]],
                        )
                        nc.vector.tensor_tensor(
                            out=oh3, in0=iota_b, in1=srl_b,
                            op=mybir.AluOpType.is_equal,
                        )
                        ohtiles[(ci, rb)] = oh
                for b in blocks:
                    mms = blk_matmuls[b]
                    if not mms:
                        ot = opool.tile([P, d], mybir.dt.float32)
                        nc.vector.memset(ot[:], 0.0)
                        nc.sync.dma_start(out_ap[b * P : (b + 1) * P, :], ot[:])
                        continue
                    ps = ppool.tile([P, d], mybir.dt.float32, space="PSUM")
                    for i, (ci, t, rb) in enumerate(mms):
                        ohfull = ohtiles[(ci, rb)][:, :]
                        lhsT = bass.AP(
                            ohfull.tensor, ohfull.offset + t,
                            [ohfull.ap[0], [call_cap, P]],
                        )
                        nc.tensor.matmul(
                            ps[:],
                            lhsT=lhsT,
                            rhs=gtiles[ci][:, t * d : (t + 1) * d],
                            start=(i == 0),
                            stop=(i == len(mms) - 1),
                        )
                    sq = spool.tile([P, d], mybir.dt.float32)
                    ss = spool.tile([P, 1], mybir.dt.float32)
                    nc.scalar.activation(
                        sq[:], ps[:], mybir.ActivationFunctionType.Square,
                        accum_out=ss[:],
                    )
                    nrm = spool.tile([P, 1], mybir.dt.float32)
                    nc.scalar.activation(
                        nrm[:], ss[:], mybir.ActivationFunctionType.Sqrt
                    )
                    nc.vector.tensor_scalar(
                        out=nrm[:], in0=nrm[:], scalar1=1e-12, scalar2=None,
                        op0=mybir.AluOpType.max,
                    )
                    nc.vector.reciprocal(nrm[:], nrm[:])
                    ot = opool.tile([P, d], mybir.dt.float32)
                    nc.scalar.activation(
                        ot[:], ps[:], mybir.ActivationFunctionType.Copy,
                        scale=nrm[:],
                    )
                    nc.sync.dma_start(out_ap[b * P : (b + 1) * P, :], ot[:])

        if hw_loop and repeats > 1:
            assert repeats % hw_loop == 0
            with tc.For_i(0, repeats // hw_loop) as _i:
                for _u in range(hw_loop):
                    emit_body()
        else:
            for _rep in range(repeats):
                emit_body()

    nc.compile()
    return nc


_PROGRAM_CACHE = {}


def _get_program(capsub, layout, **kw):
    key = (capsub.tobytes(), tuple(sorted(kw.items())))
    if key not in _PROGRAM_CACHE:
        _PROGRAM_CACHE[key] = build_program(capsub, layout, **kw)
    return _PROGRAM_CACHE[key]


def kernel(**inputs):
    emb = inputs["emb"]
    mask = inputs["mask"]
    in_maps, capsub, layout = preprocess(emb, mask)
    nc = _get_program(capsub, layout)

    from concourse.bass_utils import run_bass_kernel_spmd

    res = run_bass_kernel_spmd(nc, in_maps, core_ids=list(range(NCORES)))
    out = np.empty((N_SPOT, D), np.float32)
    for k in range(NCORES):
        out[k * SEG_PER_CORE : (k + 1) * SEG_PER_CORE] = res.results[k]["out"][
            :SEG_PER_CORE
        ]
    return out
